# revision 1
# baseline (speedup 1.0000x reference)
"""CRF Viterbi decode (forward max-plus scan + backpointer backtrace + one-hot)
on 8 Trainium2 NeuronCores, data-parallel over the batch axis.

Host contract: kernel(x[256,1024,128] f32, transitions[128,128] f32,
seq_lens[256] i32) -> one_hot(tags)[256,1024,128] f32, bit-matching the jax
reference (first-index argmax tie-breaking).

Per-core layout (BLOC=32 batches as SBUF partitions 0..31, C=128 classes):
  forward step t: for each batch b, scores = Tt + broadcast(alpha[b,:]) is
  computed by a fused tensor_tensor_reduce (add + running max -> m) on DVE;
  the PE broadcasts alpha rows into PSUM via K=1 ones-matmuls (exact fp32);
  max_index extracts the first-argmax backpointer column; backpointers are
  transposed (PE) into [b, j] rows and streamed to DRAM. Sequence-length
  masking is data-driven via copy_predicated so one SPMD program serves all
  cores; lanes are globally sorted by seq_len so a phase schedule (shrinking
  active-lane count) compacts the work.
"""

import sys

sys.path.insert(0, "/opt/trn_rl_repo")

import numpy as np

B, T, C = 256, 1024, 128
NCORES = 8
BLOC = B // NCORES  # 32

_prog_cache = {}


def _assign_lanes(seq_lens: np.ndarray) -> np.ndarray:
    """Snake-deal batches (sorted by seq_len desc) to cores -> [NCORES, BLOC]."""
    order = np.argsort(-seq_lens, kind="stable")
    lanes = [[] for _ in range(NCORES)]
    for i, b in enumerate(order):
        r, k = divmod(i, NCORES)
        c = k if r % 2 == 0 else NCORES - 1 - k
        lanes[c].append(int(b))
    return np.array(lanes, dtype=np.int64)


def _phase_schedule(seq_lens, lanes, ct, t_len, bloc):
    """Per chunk of CT steps, the max (over cores) active-lane count, merged
    into maximal runs [(chunk_lo, chunk_hi, n), ...]."""
    nch = (t_len + ct - 1) // ct
    L = seq_lens[lanes]  # [NCORES, BLOC] descending per row
    ns = []
    for cidx in range(nch):
        t0 = cidx * ct
        n = int(max((row > t0).sum() for row in L))
        n = max(n, 1)
        # quantize up to a few levels so phases span many chunks (keeps the
        # traced instruction count down; For_i loops the chunks of a phase)
        q = bloc // 4 if bloc >= 4 else 1
        n = min(bloc, ((n + q - 1) // q) * q)
        ns.append(n)
    phases = []
    lo = 0
    for cidx in range(1, nch + 1):
        if cidx == nch or ns[cidx] != ns[lo]:
            phases.append((lo, cidx, ns[lo]))
            lo = cidx
    return phases


def build_program(t_len=T, ct=64, bloc=BLOC, phases=None, gs=8, trn="TRN2",
                  num_devices=NCORES):
    """Build the SPMD bass program. Returns (nc, meta)."""
    import concourse.bass as bass
    import concourse.bacc as bacc
    import concourse.mybir as mybir
    from concourse import tile

    f32 = mybir.dt.float32
    u32 = mybir.dt.uint32
    i8 = mybir.dt.int8
    Alu = mybir.AluOpType
    nch = (t_len + ct - 1) // ct
    assert nch * ct == t_len
    if phases is None:
        phases = [(0, nch, bloc)]

    nc = bacc.Bacc(trn, target_bir_lowering=False, debug=False,
                   num_devices=num_devices)

    TC = t_len * C
    x_d = nc.dram_tensor("x", [bloc, TC], f32, kind="ExternalInput").ap()
    tt_d = nc.dram_tensor("tt", [C, C], f32, kind="ExternalInput").ap()
    esel_d = nc.dram_tensor("esel", [bloc, bloc * C], f32, kind="ExternalInput").ap()
    ident_d = nc.dram_tensor("ident", [C, C], f32, kind="ExternalInput").ap()
    iota_d = nc.dram_tensor("iota", [bloc, C], f32, kind="ExternalInput").ap()
    mask_d = nc.dram_tensor("mask", [bloc, t_len], i8, kind="ExternalInput").ap()
    bp_d = nc.dram_tensor("bp", [bloc, TC], f32)  # internal DRAM scratch
    out_d = nc.dram_tensor("out", [bloc, TC], f32, kind="ExternalOutput").ap()

    # persistent SBUF
    tt_sb = nc.alloc_sbuf_tensor("tt_sb", [C, C], f32).ap()
    esel_sb = nc.alloc_sbuf_tensor("esel_sb", [bloc, bloc * C], f32).ap()
    ident_sb = nc.alloc_sbuf_tensor("ident_sb", [C, C], f32).ap()
    iota_sb = nc.alloc_sbuf_tensor("iota_sb", [bloc, C], f32).ap()
    alpha = nc.alloc_sbuf_tensor("alpha", [bloc, C], f32).ap()
    m_t = nc.alloc_sbuf_tensor("m_t", [C, bloc], f32)
    bpu_t = nc.alloc_sbuf_tensor("bpu_t", [C, 8 * bloc], u32)
    bpf = nc.alloc_sbuf_tensor("bpf", [C, bloc], f32).ap()
    tagv = nc.alloc_sbuf_tensor("tagv", [bloc, t_len], f32).ap()
    xr = [nc.alloc_sbuf_tensor(f"xr{p}", [bloc, ct * C], f32).ap() for p in (0, 1)]
    br = [nc.alloc_sbuf_tensor(f"br{p}", [bloc, ct * C], f32).ap() for p in (0, 1)]
    mr = [nc.alloc_sbuf_tensor(f"mr{p}", [bloc, ct], i8).ap() for p in (0, 1)]

    m_ap = m_t.ap()
    bpu_ap = bpu_t.ap()

    def bcast(ap, dim, n):
        """Insert a step-0 (broadcast) dim of size n at position dim."""
        a = ap[tuple(slice(None) for _ in ap.shape)]
        a.ap.insert(dim, [0, n])
        return a

    def m8(b):
        # m_t[:, b] broadcast to free size 8 (for max_index's in_max)
        return bass.AP(m_t, b, [[bloc, C], [0, 8]])

    def bpu_col0():
        # column 0 of each 8-wide slot: [C, bloc] u32 view
        return bass.AP(bpu_t, 0, [[8 * bloc, C], [8, bloc]])

    def mask_col(ring, s, n):
        # mask ring column s broadcast along free C -> [n, C]
        return bass.AP(ring.tensor, ring.offset + s, [[ct, n], [0, C]])

    with tile.TileContext(nc) as tc:
        with (
            tc.tile_pool(name="psA", bufs=2, space="PSUM") as psA,
            tc.tile_pool(name="psS", bufs=2, space="PSUM") as psS,
            tc.tile_pool(name="sbA", bufs=2) as sbA,
            tc.tile_pool(name="sc", bufs=4) as scp,
        ):
            # one-time loads
            nc.sync.dma_start(out=tt_sb, in_=tt_d)
            nc.sync.dma_start(out=esel_sb, in_=esel_d)
            nc.sync.dma_start(out=ident_sb, in_=ident_d)
            nc.sync.dma_start(out=iota_sb, in_=iota_d)
            nc.sync.dma_start(out=alpha, in_=x_d[:, 0:C])  # alpha0 = x[:,0,:]

            def chunk_body(iv, p, n, first_skip):
                """Forward chunk: steps s=0..ct-1 of chunk iv (parity p).
                n = active lanes. first_skip: skip s==0 (t==0) in chunk 0."""
                nc.sync.dma_start(out=xr[p], in_=x_d[:, bass.ds(iv * ct * C, ct * C)])
                nc.sync.dma_start(out=mr[p], in_=mask_d[:, bass.ds(iv * ct, ct)])
                # identity prefill of bp ring (masked steps keep iota rows)
                nc.scalar.copy(out=br[p].rearrange("b (s c) -> b s c", c=C),
                               in_=bcast(iota_sb, 1, ct))
                ng = (n + gs - 1) // gs
                for s in range(ct):
                    if first_skip and s == 0:
                        continue
                    for g in range(ng):
                        b0, b1 = g * gs, min(n, (g + 1) * gs)
                        nb = b1 - b0
                        abuf = psA.tile([C, gs * C], f32, tag="abuf")
                        for k in range(nb):
                            b = b0 + k
                            nc.tensor.matmul(
                                abuf[:, k * C:(k + 1) * C],
                                esel_sb[:, b * C:(b + 1) * C], alpha,
                                start=True, stop=True,
                            )
                        asb = sbA.tile([C, gs * C], f32, tag="asb")
                        nc.scalar.copy(out=asb[:, 0:nb * C], in_=abuf[:, 0:nb * C])
                        sc = scp.tile([C, gs * C], f32, tag="sc")
                        nc.vector.tensor_tensor(
                            out=sc[:, 0:nb * C].rearrange("j (k c) -> j k c", c=C),
                            in0=bcast(tt_sb, 1, nb),
                            in1=asb[:, 0:nb * C].rearrange("j (k c) -> j k c", c=C),
                            op=Alu.add)
                        nc.vector.tensor_reduce(
                            out=m_ap[:, b0:b1],
                            in_=sc[:, 0:nb * C].rearrange("j (k c) -> j k c", c=C),
                            axis=mybir.AxisListType.X, op=Alu.max)
                        for k in range(nb):
                            b = b0 + k
                            nc.vector.max_index(
                                out=bpu_ap[:, b * 8:(b + 1) * 8],
                                in_max=m8(b), in_values=sc[:, k * C:(k + 1) * C],
                            )
                    # bp column extract + transposes + state update
                    nc.vector.tensor_copy(out=bpf[:, 0:n], in_=bass.AP(bpu_t, 0, [[8 * bloc, C], [8, n]]))
                    tpA = psS.tile([bloc, C], f32, tag="tpA")
                    nc.tensor.transpose(tpA[:], m_ap, ident_sb)
                    tpB = psS.tile([bloc, C], f32, tag="tpB")
                    nc.tensor.transpose(tpB[:], bpf, ident_sb)
                    sc2 = scp.tile([bloc, C], f32, tag="sc2")
                    nc.vector.tensor_tensor(
                        out=sc2[0:n, :], in0=tpA[0:n, :],
                        in1=xr[p][0:n, s * C:(s + 1) * C], op=Alu.add)
                    nc.vector.copy_predicated(
                        out=alpha[0:n, :], mask=mask_col(mr[p], s, n),
                        data=sc2[0:n, :])
                    nc.vector.copy_predicated(
                        out=br[p][0:n, s * C:(s + 1) * C],
                        mask=mask_col(mr[p], s, n), data=tpB[0:n, :])
                nc.sync.dma_start(out=bp_d.ap()[:, bass.ds(iv * ct * C, ct * C)],
                                  in_=br[p])

            # ---- forward phases ----
            for (lo, hi, n) in phases:
                def mk(nn, is_first):
                    def body2(iv0, unroll):
                        for u in range(unroll):
                            chunk_body(iv0 + u, u % 2, nn,
                                       first_skip=(is_first and u == 0))
                    return body2
                if lo == 0:
                    # chunk 0 traced alone (skips t=0)
                    tc.For_i_unrolled_general(
                        start=0, end=1, step=1,
                        unrollable_body=mk(n, True), max_unroll=1)
                    if hi > 1:
                        tc.For_i_unrolled_general(
                            start=1, end=hi, step=1,
                            unrollable_body=mk(n, False), max_unroll=1)
                else:
                    tc.For_i_unrolled_general(
                        start=lo, end=hi, step=1,
                        unrollable_body=mk(n, False), max_unroll=1)

            # ---- last tag ----
            mx = scp.tile([bloc, 1], f32, tag="mx")
            nc.vector.tensor_reduce(out=mx[:], in_=alpha, axis=mybir.AxisListType.X,
                                    op=Alu.max)
            lt8 = scp.tile([bloc, 8], u32, tag="lt8")
            mx8 = bass.AP(mx.tensor, mx.offset, [[mx.ap.to_list()[0][0], bloc], [0, 8]])
            nc.vector.max_index(out=lt8[:], in_max=mx8, in_values=alpha)
            nc.vector.tensor_copy(out=tagv[:, t_len - 1:t_len], in_=lt8[:, 0:1])

            # ---- backtrace + one-hot (static chunk loop, reversed) ----
            for cc in range(nch - 1, -1, -1):
                p = cc % 2
                nc.sync.dma_start(out=xr[p], in_=bp_d.ap()[:, cc * ct * C:(cc + 1) * ct * C])
                for s in range(ct - 1, -1, -1):
                    u = cc * ct + s
                    if u == 0:
                        continue
                    oh = scp.tile([bloc, C], f32, tag="oh")
                    nc.vector.tensor_scalar(
                        out=oh[:], in0=iota_sb, scalar1=tagv[:, u:u + 1],
                        scalar2=None, op0=Alu.is_equal)
                    dot = scp.tile([bloc, C], f32, tag="dot")
                    nc.vector.scalar_tensor_tensor(
                        out=dot[:], in0=oh[:], scalar=1.0,
                        in1=xr[p][:, s * C:(s + 1) * C],
                        op0=Alu.mult, op1=Alu.mult,
                        accum_out=tagv[:, u - 1:u])
                # one-hot emit for chunk cc (tags for its steps are final)
                ohc = br[p]
                nc.vector.tensor_tensor(
                    out=ohc.rearrange("b (s c) -> b s c", c=C),
                    in0=bcast(iota_sb, 1, ct),
                    in1=bcast(tagv[:, cc * ct:(cc + 1) * ct], 2, C),
                    op=Alu.is_equal)
                nc.sync.dma_start(out=out_d[:, cc * ct * C:(cc + 1) * ct * C],
                                  in_=ohc)

    nc.compile()
    return nc


def _host_inputs(x, transitions, seq_lens, lanes, t_len=T, bloc=BLOC):
    """Build per-core input maps."""
    tt = np.ascontiguousarray(transitions.T).astype(np.float32)
    esel = np.zeros((bloc, bloc * C), np.float32)
    for b in range(bloc):
        esel[b, b * C:(b + 1) * C] = 1.0
    ident = np.eye(C, dtype=np.float32)
    iota = np.tile(np.arange(C, dtype=np.float32), (bloc, 1))
    in_maps = []
    for c in range(lanes.shape[0]):
        lx = x[lanes[c]][:, :t_len, :].reshape(bloc, t_len * C).astype(np.float32)
        L = seq_lens[lanes[c]].astype(np.int64)
        tgrid = np.arange(t_len)[None, :]
        mask = (tgrid < L[:, None]).astype(np.int8)  # active at step t: t < L
        in_maps.append({
            "x": np.ascontiguousarray(lx),
            "tt": tt, "esel": esel, "ident": ident, "iota": iota,
            "mask": np.ascontiguousarray(mask),
        })
    return in_maps


TRACE = False
LAST_RESULT = None


def kernel(x, transitions, seq_lens):
    global LAST_RESULT
    from concourse.bass_utils import run_bass_kernel_spmd

    x = np.asarray(x, dtype=np.float32)
    transitions = np.asarray(transitions, dtype=np.float32)
    seq_lens = np.asarray(seq_lens)
    lanes = _assign_lanes(seq_lens)
    ct = 64
    phases = _phase_schedule(seq_lens, lanes, ct, T, BLOC)
    key = tuple(phases)
    if key not in _prog_cache:
        _prog_cache[key] = build_program(T, ct, BLOC, phases)
    nc = _prog_cache[key]
    in_maps = _host_inputs(x, transitions, seq_lens, lanes)
    res = run_bass_kernel_spmd(nc, in_maps, list(range(NCORES)), trace=TRACE)
    LAST_RESULT = res
    out = np.empty((B, T, C), np.float32)
    for c in range(NCORES):
        out[lanes[c]] = res.results[c]["out"].reshape(BLOC, T, C)
    return out



# revision 4
# speedup vs baseline: 1.2844x; 1.2844x over previous
"""CRF Viterbi decode (forward max-plus scan + backpointer backtrace + one-hot)
on 8 Trainium2 NeuronCores, data-parallel over the batch axis.

v2 design (vs baseline: 19.9ms):
- Forward scores+max fused into ONE custom DVE op (VITERBI_SCORE_MAX:
  out = in0+in1, accum_out = max-reduce) registered at runtime in the
  per-NEFF DVE table; argmax backpointers via stock max_index.
- 4 independent lane-groups of 8 (stride-4 interleave: group g = sorted
  lanes {g, g+4, ..., g+28}) pipelined against each other so the per-step
  serial chain (PE broadcast -> Act copy -> DVE -> PE transpose -> Pool add
  -> DVE masked update) never drains any engine; groups stay ~balanced in
  active-lane count at every phase.
- Pool (GpSimd) engine does the br-ring identity prefills (iota) and the
  potential-add tail; Act does PSUM->SBUF staging; PE does the alpha row
  broadcasts (K=8 one-hot matmuls) and the [C,8]->[8,C] transposes.
- Backward gather fused to ONE scalar_tensor_tensor per step
  (tag[u-1] = sum_i (iota==tag_u)*bp_u[i]); one-hot emit per chunk on DVE.
- Sequence-length masking via copy_predicated + identity-prefilled bp rings;
  lanes globally sorted by length, phase schedule shrinks active lanes.
"""

import sys

sys.path.insert(0, "/opt/trn_rl_repo")

import numpy as np

B, T, C = 256, 1024, 128
NCORES = 8
BLOC = B // NCORES   # 32 lanes per core
NG = 4               # lane groups per core
GL = BLOC // NG      # 8 lanes per group
CT = 8               # steps per chunk
NCH = T // CT
Q = 4                # phase quantization of active-lane count

_prog_cache = {}


def _get_vsm():
    """Register (idempotently) the fused add+max-accum custom DVE op."""
    from concourse import dve_ops
    from concourse.dve_spec import Spec, Src0, Src1, AluOp, lower as dve_lower
    from concourse.dve_uop import DveOpSpec

    name = "VITERBI_SCORE_MAX"
    for op in dve_ops.OPS:
        if op.name == name:
            return op

    def _ref(in0, in1, s0, s1, imm2):
        b = (in0.astype(np.float32) + in1).astype(np.float32)
        return b, b.reshape(b.shape[0], -1).max(axis=-1, keepdims=True)

    spec = Spec(body=Src0 + Src1, accum=AluOp.MAX, reference=_ref)
    row = dve_ops._CUSTOM_DVE_ROW_BASE + len(dve_ops.OPS)
    assert row < 0x20
    shas = {}
    for ver in ("v3", "v4"):
        u = dve_lower(spec, ver=ver)
        shas[ver] = DveOpSpec(name=name, opcode=row, uops=u, rd1_en=True).sha(ver)
    op = dve_ops.DveOp(name, spec, subdim=False, uops_sha=shas)
    dve_ops.OPS.append(op)
    dve_ops._SUB_OPCODE_FOR_NAME[name] = row
    dve_ops.CUSTOM_DVE_SPECS[name] = spec
    return op


def _assign_lanes(seq_lens: np.ndarray) -> np.ndarray:
    """Snake-deal batches (sorted by seq_len desc) to cores -> [NCORES, BLOC].
    lanes[c][p] = batch at sorted-position p on core c (p=0 longest)."""
    order = np.argsort(-seq_lens, kind="stable")
    lanes = [[] for _ in range(NCORES)]
    for i, b in enumerate(order):
        r, k = divmod(i, NCORES)
        c = k if r % 2 == 0 else NCORES - 1 - k
        lanes[c].append(int(b))
    return np.array(lanes, dtype=np.int64)


def _phase_schedule(seq_lens, lanes):
    """Per chunk, max (over cores) active-lane count, quantized up to Q,
    merged into maximal runs [(chunk_lo, chunk_hi, n), ...]."""
    L = seq_lens[lanes]  # [NCORES, BLOC] descending per row
    ns = []
    for cidx in range(NCH):
        t0 = cidx * CT
        n = int(max((row > t0).sum() for row in L))
        n = max(n, 1)
        n = min(BLOC, ((n + Q - 1) // Q) * Q)
        ns.append(n)
    phases = []
    lo = 0
    for cidx in range(1, NCH + 1):
        if cidx == NCH or ns[cidx] != ns[lo]:
            phases.append((lo, cidx, ns[lo]))
            lo = cidx
    return phases


def build_program(phases, trn="TRN2", num_devices=NCORES):
    import concourse.bass as bass
    import concourse.bacc as bacc
    import concourse.mybir as mybir
    from concourse import tile

    vsm = _get_vsm()

    f32 = mybir.dt.float32
    u32 = mybir.dt.uint32
    i8 = mybir.dt.int8
    Alu = mybir.AluOpType
    TC = T * C

    nc = bacc.Bacc(trn, target_bir_lowering=False, debug=False,
                   num_devices=num_devices)

    # ---- DRAM ----
    x_d = [nc.dram_tensor(f"x{g}", [GL, TC], f32, kind="ExternalInput").ap()
           for g in range(NG)]
    mk_d = [nc.dram_tensor(f"mask{g}", [GL, T], i8, kind="ExternalInput").ap()
            for g in range(NG)]
    tt_d = nc.dram_tensor("tt", [C, C], f32, kind="ExternalInput").ap()
    esel_d = nc.dram_tensor("esel", [GL, GL * C], f32, kind="ExternalInput").ap()
    ident_d = nc.dram_tensor("ident", [C, C], f32, kind="ExternalInput").ap()
    iota32_d = nc.dram_tensor("iota32", [BLOC, C], f32, kind="ExternalInput").ap()
    bp_d = [nc.dram_tensor(f"bp{g}", [GL, TC], f32) for g in range(NG)]
    lt_d = nc.dram_tensor("lt", [BLOC, 1], f32)
    out_d = nc.dram_tensor("out", [BLOC, TC], f32, kind="ExternalOutput").ap()

    # ---- persistent SBUF ----
    tt_sb = nc.alloc_sbuf_tensor("tt_sb", [C, C], f32).ap()
    esel_sb = nc.alloc_sbuf_tensor("esel_sb", [GL, GL * C], f32).ap()
    ident_sb = nc.alloc_sbuf_tensor("ident_sb", [C, C], f32).ap()
    iota32_sb = nc.alloc_sbuf_tensor("iota32_sb", [BLOC, C], f32).ap()
    alpha = [nc.alloc_sbuf_tensor(f"alpha{g}", [GL, C], f32).ap() for g in range(NG)]
    mb = [nc.alloc_sbuf_tensor(f"mb{g}", [C, 2 * GL], f32) for g in range(NG)]
    bpu = [nc.alloc_sbuf_tensor(f"bpu{g}", [C, 8 * GL], u32) for g in range(NG)]
    sc = [nc.alloc_sbuf_tensor(f"sc{g}", [C, GL * C], f32).ap() for g in range(NG)]
    sc2 = [nc.alloc_sbuf_tensor(f"sc2_{g}", [GL, C], f32).ap() for g in range(NG)]
    ltf = [nc.alloc_sbuf_tensor(f"ltf{g}", [GL, 8], u32) for g in range(NG)]
    mxg = [nc.alloc_sbuf_tensor(f"mxg{g}", [GL, 1], f32) for g in range(NG)]
    xr = [[nc.alloc_sbuf_tensor(f"xr{g}_{p}", [GL, CT * C], f32).ap() for p in (0, 1)]
          for g in range(NG)]
    br = [[nc.alloc_sbuf_tensor(f"br{g}_{p}", [GL, CT * C], f32).ap() for p in (0, 1)]
          for g in range(NG)]
    mr = [[nc.alloc_sbuf_tensor(f"mr{g}_{p}", [GL, CT], i8) for p in (0, 1)]
          for g in range(NG)]
    tagv = nc.alloc_sbuf_tensor("tagv", [BLOC, T], f32).ap()
    ohb = nc.alloc_sbuf_tensor("ohb", [BLOC, C], f32).ap()
    xb = [nc.alloc_sbuf_tensor(f"xb{p}", [BLOC, CT * C], f32).ap() for p in (0, 1)]
    oe = [nc.alloc_sbuf_tensor(f"oe{p}", [BLOC, CT * C], f32).ap() for p in (0, 1)]

    def bcast(ap, dim, n):
        a = ap[tuple(slice(None) for _ in ap.shape)]
        a.ap.insert(dim, [0, n])
        return a

    def m8(g, r):
        # mb[g][:, r] broadcast to free size 8 (max_index in_max)
        return bass.AP(mb[g], r, [[2 * GL, C], [0, 8]])

    def bpu_col0(g, nb):
        return bass.AP(bpu[g], 0, [[8 * GL, C], [8, nb]])

    def mask_col(g, p, s, nb):
        return bass.AP(mr[g][p], s, [[CT, nb], [0, C]])

    def nb_of(g, n):
        # active rows in group g at phase width n (group rows r -> lane 4r+g)
        return max(0, min(GL, (n - g + NG - 1) // NG))

    with tile.TileContext(nc) as tc:
        with (
            tc.tile_pool(name="psA", bufs=2, space="PSUM") as psA,
            tc.tile_pool(name="psT", bufs=2, space="PSUM") as psT,
            tc.tile_pool(name="sbA", bufs=2) as sbA,
            tc.tile_pool(name="sbT", bufs=2) as sbT,
        ):
            # one-time loads
            nc.sync.dma_start(out=tt_sb, in_=tt_d)
            nc.sync.dma_start(out=esel_sb, in_=esel_d)
            nc.sync.dma_start(out=ident_sb, in_=ident_d)
            nc.sync.dma_start(out=iota32_sb, in_=iota32_d)
            for g in range(NG):
                nc.sync.dma_start(out=alpha[g], in_=x_d[g][:, 0:C])

            def emit_S(g, p, s, nb):
                """scores+max+argmax block for group g, step s (parity p)."""
                abuf = psA.tile([C, GL * C], f32, tag="abuf")
                for r in range(nb):
                    nc.tensor.matmul(
                        abuf[:, r * C:(r + 1) * C],
                        esel_sb[:, r * C:(r + 1) * C], alpha[g],
                        start=True, stop=True)
                asb = sbA.tile([C, GL * C], f32, tag=f"asb{g}")
                nc.scalar.copy(out=asb[:, 0:nb * C], in_=abuf[:, 0:nb * C])
                for r in range(nb):
                    nc.vector._custom_dve(
                        vsm,
                        out=sc[g][:, r * C:(r + 1) * C],
                        in0=tt_sb, in1=asb[:, r * C:(r + 1) * C],
                        accum_out=bass.AP(mb[g], r, [[2 * GL, C], [1, 1]]))
                for r in range(nb):
                    nc.vector.max_index(
                        out=bass.AP(bpu[g], 8 * r, [[8 * GL, C], [1, 8]]),
                        in_max=m8(g, r),
                        in_values=sc[g][:, r * C:(r + 1) * C])
                nc.vector.tensor_copy(
                    out=bass.AP(mb[g], GL, [[2 * GL, C], [1, nb]]),
                    in_=bpu_col0(g, nb))

            def emit_P(g, p, s, nb):
                """transpose + potential add + masked alpha/bp update, group g."""
                tpm = psT.tile([GL, C], f32, tag="tpm")
                nc.tensor.transpose(
                    tpm[:], bass.AP(mb[g], 0, [[2 * GL, C], [1, GL]]), ident_sb)
                tpb = psT.tile([GL, C], f32, tag="tpb")
                nc.tensor.transpose(
                    tpb[:], bass.AP(mb[g], GL, [[2 * GL, C], [1, GL]]), ident_sb)
                tpms = sbT.tile([GL, C], f32, tag="tpms")
                nc.scalar.copy(out=tpms[:], in_=tpm[:])
                tpbs = sbT.tile([GL, C], f32, tag="tpbs")
                nc.scalar.copy(out=tpbs[:], in_=tpb[:])
                # potential add on Pool (GpSimd)
                nc.gpsimd.tensor_tensor(
                    out=sc2[g][0:nb, :], in0=tpms[0:nb, :],
                    in1=xr[g][p][0:nb, s * C:(s + 1) * C], op=Alu.add)
                nc.vector.copy_predicated(
                    out=alpha[g][0:nb, :], mask=mask_col(g, p, s, nb),
                    data=sc2[g][0:nb, :])
                nc.vector.copy_predicated(
                    out=br[g][p][0:nb, s * C:(s + 1) * C],
                    mask=mask_col(g, p, s, nb), data=tpbs[0:nb, :])

            def chunk_body(iv, p, n, first_skip):
                groups = [g for g in range(NG) if nb_of(g, n) > 0]
                for g in groups:
                    nc.sync.dma_start(out=xr[g][p],
                                      in_=x_d[g][:, bass.ds(iv * CT * C, CT * C)])
                    nc.sync.dma_start(out=mr[g][p].ap(),
                                      in_=mk_d[g][:, bass.ds(iv * CT, CT)])
                    # identity prefill of bp ring on Pool (iota over class dim)
                    nc.gpsimd.iota(
                        out=br[g][p].rearrange("b (s c) -> b s c", c=C),
                        pattern=[[0, CT], [1, C]], base=0, channel_multiplier=0,
                        allow_small_or_imprecise_dtypes=True)
                for s in range(CT):
                    if first_skip and s == 0:
                        continue
                    pend = []
                    for g in groups:
                        nb = nb_of(g, n)
                        emit_S(g, p, s, nb)
                        pend.append((g, nb))
                        if len(pend) >= 2:
                            pg, pnb = pend.pop(0)
                            emit_P(pg, p, s, pnb)
                    for (pg, pnb) in pend:
                        emit_P(pg, p, s, pnb)
                for g in groups:
                    nc.sync.dma_start(
                        out=bp_d[g].ap()[:, bass.ds(iv * CT * C, CT * C)],
                        in_=br[g][p])

            # ---- forward phases ----
            for (lo, hi, n) in phases:
                def mk(nn, is_first):
                    def body2(iv0, unroll):
                        for u in range(unroll):
                            chunk_body(iv0 + u, u % 2, nn,
                                       first_skip=(is_first and u == 0))
                    return body2
                if lo == 0:
                    tc.For_i_unrolled_general(
                        start=0, end=1, step=1,
                        unrollable_body=mk(n, True), max_unroll=1)
                    if hi > 1:
                        tc.For_i_unrolled_general(
                            start=1, end=hi, step=1,
                            unrollable_body=mk(n, False), max_unroll=2)
                else:
                    tc.For_i_unrolled_general(
                        start=lo, end=hi, step=1,
                        unrollable_body=mk(n, False), max_unroll=2)

            # ---- last tag (per group, bounced через DRAM to tagv rows) ----
            for g in range(NG):
                nc.vector.tensor_reduce(
                    out=mxg[g].ap(), in_=alpha[g], axis=mybir.AxisListType.X,
                    op=Alu.max)
                nc.vector.max_index(
                    out=ltf[g].ap(),
                    in_max=bass.AP(mxg[g], 0, [[1, GL], [0, 8]]),
                    in_values=alpha[g])
                # u32 -> f32 convert into sc2 scratch col, then DMA to DRAM
                nc.vector.tensor_copy(
                    out=sc2[g][:, 0:1], in_=bass.AP(ltf[g], 0, [[8, GL], [1, 1]]))
                nc.sync.dma_start(out=lt_d.ap()[8 * g:8 * (g + 1), :],
                                  in_=sc2[g][:, 0:1])
            nc.sync.dma_start(out=tagv[:, T - 1:T], in_=lt_d.ap())

            # ---- backtrace + one-hot emit (reversed chunks) ----
            for cc in range(NCH - 1, -1, -1):
                p = cc % 2
                for g in range(NG):
                    nc.sync.dma_start(
                        out=xb[p][8 * g:8 * (g + 1), :],
                        in_=bp_d[g].ap()[:, cc * CT * C:(cc + 1) * CT * C])
                for s in range(CT - 1, -1, -1):
                    u = cc * CT + s
                    if u == 0:
                        continue
                    nc.vector.scalar_tensor_tensor(
                        out=ohb, in0=iota32_sb, scalar=tagv[:, u:u + 1],
                        in1=xb[p][:, s * C:(s + 1) * C],
                        op0=Alu.is_equal, op1=Alu.mult,
                        accum_out=tagv[:, u - 1:u])
                nc.vector.tensor_tensor(
                    out=oe[p].rearrange("b (s c) -> b s c", c=C),
                    in0=bcast(iota32_sb, 1, CT),
                    in1=bcast(tagv[:, cc * CT:(cc + 1) * CT], 2, C),
                    op=Alu.is_equal)
                nc.sync.dma_start(out=out_d[:, cc * CT * C:(cc + 1) * CT * C],
                                  in_=oe[p])

    nc.compile()
    return nc


def _host_inputs(x, transitions, seq_lens, lanes):
    tt = np.ascontiguousarray(transitions.T).astype(np.float32)
    esel = np.zeros((GL, GL * C), np.float32)
    for r in range(GL):
        esel[r, r * C:(r + 1) * C] = 1.0
    ident = np.eye(C, dtype=np.float32)
    iota32 = np.tile(np.arange(C, dtype=np.float32), (BLOC, 1))
    tgrid = np.arange(T)[None, :]
    in_maps = []
    for c in range(lanes.shape[0]):
        im = {"tt": tt, "esel": esel, "ident": ident, "iota32": iota32}
        for g in range(NG):
            rows = [lanes[c][NG * r + g] for r in range(GL)]
            lx = x[rows].reshape(GL, T * C).astype(np.float32)
            L = seq_lens[rows].astype(np.int64)
            im[f"x{g}"] = np.ascontiguousarray(lx)
            im[f"mask{g}"] = np.ascontiguousarray(
                (tgrid < L[:, None]).astype(np.int8))
        in_maps.append(im)
    return in_maps


TRACE = False
LAST_RESULT = None


def kernel(x, transitions, seq_lens):
    global LAST_RESULT
    from concourse.bass_utils import run_bass_kernel_spmd

    x = np.asarray(x, dtype=np.float32)
    transitions = np.asarray(transitions, dtype=np.float32)
    seq_lens = np.asarray(seq_lens)
    lanes = _assign_lanes(seq_lens)
    phases = _phase_schedule(seq_lens, lanes)
    key = tuple(phases)
    if key not in _prog_cache:
        _prog_cache[key] = build_program(phases)
    nc = _prog_cache[key]
    in_maps = _host_inputs(x, transitions, seq_lens, lanes)
    res = run_bass_kernel_spmd(nc, in_maps, list(range(NCORES)), trace=TRACE)
    LAST_RESULT = res
    out = np.empty((B, T, C), np.float32)
    for c in range(NCORES):
        oc = res.results[c]["out"].reshape(BLOC, T, C)
        for g in range(NG):
            for r in range(GL):
                out[lanes[c][NG * r + g]] = oc[GL * g + r]
    return out


# revision 17
# speedup vs baseline: 1.5154x; 1.1799x over previous
"""CRF Viterbi decode (forward max-plus scan + backpointer backtrace + one-hot)
on 8 Trainium2 NeuronCores, data-parallel over the batch axis.

v4 design (baseline 19.9ms -> v2 15.5ms -> here):
- The per-lane state is kept TRANSPOSED: mbx_g [C, 8] = alpha^T for the 8
  lanes of group g. The per-step alpha broadcast is ONE transpose-mode
  matmul per lane of a 0-stride column-replicated view of mbx (2cyc/row
  fp32, ~294ns — vs 523ns for the v2 K=8 esel matmul), and the masked
  alpha update shrinks to free-size-8 DVE ops (add+copy_predicated on
  [C, nb] in the transposed domain, ~68ns each).
- Forward scores+max fused into ONE custom DVE op (VITERBI_SCORE_MAX:
  out = in0+in1, accum_out = max-reduce) registered at runtime into the
  per-NEFF DVE table; first-argmax backpointers via stock max_index.
- 4 independent lane-groups of 8 (stride-4 interleave: group g = sorted
  lanes {g, g+4, ..., g+28}) software-pipelined against each other
  (emission order S0 S1 P0 S2 P1 S3 P2 P3) so the serial per-step chain
  (PE bcast -> Act copy -> DVE score/argmax -> update) never drains DVE.
- Pool (GpSimd) does the bp-ring identity prefills (iota) and the
  transposed potential-adds; x and masks are consumed pre-transposed
  (host prepares x^T [C, T*8] and an i8 mask replicated over classes).
- Backward gather fused to ONE scalar_tensor_tensor per step; one-hot
  emit per chunk on DVE; identity-prefilled bp rings make masked steps
  pass tags through unchanged.
"""

import sys

sys.path.insert(0, "/opt/trn_rl_repo")

import numpy as np

B, T, C = 256, 1024, 128
NCORES = 8
BLOC = B // NCORES   # 32 lanes per core
NG = 4               # lane groups per core
GL = BLOC // NG      # 8 lanes per group
CT = 8               # steps per chunk
NCH = T // CT
Q = 4                # phase quantization of active-lane count

_prog_cache = {}


def _get_vsm():
    """Register (idempotently) the fused add+max-accum custom DVE op."""
    from concourse import dve_ops
    from concourse.dve_spec import Spec, Src0, Src1, AluOp, lower as dve_lower
    from concourse.dve_uop import DveOpSpec

    name = "VITERBI_SCORE_MAX"
    for op in dve_ops.OPS:
        if op.name == name:
            return op

    def _ref(in0, in1, s0, s1, imm2):
        b = (in0.astype(np.float32) + in1).astype(np.float32)
        return b, b.reshape(b.shape[0], -1).max(axis=-1, keepdims=True)

    spec = Spec(body=Src0 + Src1, accum=AluOp.MAX, reference=_ref)
    row = dve_ops._CUSTOM_DVE_ROW_BASE + len(dve_ops.OPS)
    assert row < 0x20
    shas = {}
    for ver in ("v3", "v4"):
        u = dve_lower(spec, ver=ver)
        shas[ver] = DveOpSpec(name=name, opcode=row, uops=u, rd1_en=True).sha(ver)
    op = dve_ops.DveOp(name, spec, subdim=False, uops_sha=shas)
    dve_ops.OPS.append(op)
    dve_ops._SUB_OPCODE_FOR_NAME[name] = row
    dve_ops.CUSTOM_DVE_SPECS[name] = spec
    return op


def _assign_lanes(seq_lens: np.ndarray) -> np.ndarray:
    """Snake-deal batches (sorted by seq_len desc) to cores -> [NCORES, BLOC].
    lanes[c][p] = batch at sorted-position p on core c (p=0 longest)."""
    order = np.argsort(-seq_lens, kind="stable")
    lanes = [[] for _ in range(NCORES)]
    for i, b in enumerate(order):
        r, k = divmod(i, NCORES)
        c = k if r % 2 == 0 else NCORES - 1 - k
        lanes[c].append(int(b))
    return np.array(lanes, dtype=np.int64)


def _phase_schedule(seq_lens, lanes):
    """Per chunk, max (over cores) active-lane count, quantized up to Q,
    merged into maximal runs [(chunk_lo, chunk_hi, n), ...]."""
    L = seq_lens[lanes]
    ns = []
    for cidx in range(NCH):
        t0 = cidx * CT
        n = int(max((row > t0).sum() for row in L))
        n = max(n, 1)
        n = min(BLOC, ((n + Q - 1) // Q) * Q)
        ns.append(n)
    phases = []
    lo = 0
    for cidx in range(1, NCH + 1):
        if cidx == NCH or ns[cidx] != ns[lo]:
            phases.append((lo, cidx, ns[lo]))
            lo = cidx
    return phases


def build_program(phases, trn="TRN2", num_devices=NCORES):
    import concourse.bass as bass
    import concourse.bacc as bacc
    import concourse.mybir as mybir
    from concourse import tile

    vsm = _get_vsm()

    f32 = mybir.dt.float32
    u32 = mybir.dt.uint32
    i8 = mybir.dt.int8
    Alu = mybir.AluOpType
    TC = T * C
    TG = T * GL

    nc = bacc.Bacc(trn, target_bir_lowering=False, debug=False,
                   num_devices=num_devices)

    # ---- DRAM ----
    xT_d = [nc.dram_tensor(f"xT{g}", [C, TG], f32, kind="ExternalInput").ap()
            for g in range(NG)]
    mT_d = [nc.dram_tensor(f"mT{g}", [C, TG], i8, kind="ExternalInput").ap()
            for g in range(NG)]
    mk_d = [nc.dram_tensor(f"mask{g}", [GL, T], i8, kind="ExternalInput").ap()
            for g in range(NG)]
    tt_d = nc.dram_tensor("tt", [C, C], f32, kind="ExternalInput").ap()
    ident_d = nc.dram_tensor("ident", [C, C], f32, kind="ExternalInput").ap()
    iota32_d = nc.dram_tensor("iota32", [BLOC, C], f32, kind="ExternalInput").ap()
    bp_d = [nc.dram_tensor(f"bp{g}", [GL, TC], f32) for g in range(NG)]
    lt_d = nc.dram_tensor("lt", [BLOC, 1], f32)
    out_d = nc.dram_tensor("out", [BLOC, TC], f32, kind="ExternalOutput").ap()

    # ---- persistent SBUF ----
    tt_sb = nc.alloc_sbuf_tensor("tt_sb", [C, C], f32).ap()
    ident_sb = nc.alloc_sbuf_tensor("ident_sb", [C, C], f32).ap()
    iota32_sb = nc.alloc_sbuf_tensor("iota32_sb", [BLOC, C], f32).ap()
    mbx = [nc.alloc_sbuf_tensor(f"mbx{g}", [C, GL], f32) for g in range(NG)]
    mb = [nc.alloc_sbuf_tensor(f"mb{g}", [C, 2 * GL], f32) for g in range(NG)]
    bpu = [nc.alloc_sbuf_tensor(f"bpu{g}", [C, 8 * GL], u32) for g in range(NG)]
    sc = [nc.alloc_sbuf_tensor(f"sc{g}", [C, GL * C], f32).ap() for g in range(NG)]
    smx = [nc.alloc_sbuf_tensor(f"smx{g}", [C, GL], f32).ap() for g in range(NG)]
    afin = [nc.alloc_sbuf_tensor(f"afin{g}", [GL, C], f32).ap() for g in range(NG)]
    ltf = [nc.alloc_sbuf_tensor(f"ltf{g}", [GL, 8], u32) for g in range(NG)]
    mxg = [nc.alloc_sbuf_tensor(f"mxg{g}", [GL, 1], f32) for g in range(NG)]
    xTr = [[nc.alloc_sbuf_tensor(f"xTr{g}_{p}", [C, CT * GL], f32).ap()
            for p in (0, 1)] for g in range(NG)]
    mTr = [[nc.alloc_sbuf_tensor(f"mTr{g}_{p}", [C, CT * GL], i8)
            for p in (0, 1)] for g in range(NG)]
    br = [[nc.alloc_sbuf_tensor(f"br{g}_{p}", [GL, CT * C], f32).ap()
           for p in (0, 1)] for g in range(NG)]
    mr = [[nc.alloc_sbuf_tensor(f"mr{g}_{p}", [GL, CT], i8) for p in (0, 1)]
          for g in range(NG)]
    tagv = nc.alloc_sbuf_tensor("tagv", [BLOC, T], f32).ap()
    ohb = nc.alloc_sbuf_tensor("ohb", [BLOC, C], f32).ap()
    xb = [nc.alloc_sbuf_tensor(f"xb{p}", [BLOC, CT * C], f32).ap() for p in (0, 1)]
    oe = [nc.alloc_sbuf_tensor(f"oe{p}", [BLOC, CT * C], f32).ap() for p in (0, 1)]

    def bcast(ap, dim, n):
        a = ap[tuple(slice(None) for _ in ap.shape)]
        a.ap.insert(dim, [0, n])
        return a

    def m8(g, r):
        # mb[g][:, r] broadcast to free size 8 (max_index in_max)
        return bass.AP(mb[g], r, [[2 * GL, C], [0, 8]])

    def nb_of(g, n):
        return max(0, min(GL, (n - g + NG - 1) // NG))

    with tile.TileContext(nc) as tc:
        with (
            tc.tile_pool(name="psA", bufs=2, space="PSUM") as psA,
            tc.tile_pool(name="psT", bufs=2, space="PSUM") as psT,
            tc.tile_pool(name="sbA", bufs=2) as sbA,
            tc.tile_pool(name="sbT", bufs=2) as sbT,
        ):
            # one-time loads
            nc.sync.dma_start(out=tt_sb, in_=tt_d)
            nc.sync.dma_start(out=ident_sb, in_=ident_d)
            nc.sync.dma_start(out=iota32_sb, in_=iota32_d)
            for g in range(NG):
                # alpha0^T = x^T at t=0 (cols 0..GL-1 of xT)
                nc.sync.dma_start(out=mbx[g].ap(), in_=xT_d[g][:, 0:GL])

            def emit_S(g, p, s, nb):
                """broadcast + scores + max + argmax for group g, step s."""
                abuf = psA.tile([C, GL * C], f32, tag="abuf")
                for r in range(nb):
                    # transpose-mode broadcast: out[j, i] = mbx[i, r] for all j
                    nc.tensor.matmul(
                        abuf[:, r * C:(r + 1) * C],
                        bass.AP(mbx[g], r, [[GL, C], [0, C]]),
                        ident_sb, start=True, stop=True, is_transpose=True)
                asb = sbA.tile([C, GL * C], f32, tag=f"asb{g}")
                nc.scalar.copy(out=asb[:, 0:nb * C], in_=abuf[:, 0:nb * C])
                for r in range(nb):
                    nc.vector._custom_dve(
                        vsm,
                        out=sc[g][:, r * C:(r + 1) * C],
                        in0=tt_sb, in1=asb[:, r * C:(r + 1) * C],
                        accum_out=bass.AP(mb[g], r, [[2 * GL, C], [1, 1]]))
                for r in range(nb):
                    nc.vector.max_index(
                        out=bass.AP(bpu[g], 8 * r, [[8 * GL, C], [1, 8]]),
                        in_max=m8(g, r),
                        in_values=sc[g][:, r * C:(r + 1) * C])
                nc.vector.tensor_copy(
                    out=bass.AP(mb[g], GL, [[2 * GL, C], [1, nb]]),
                    in_=bass.AP(bpu[g], 0, [[8 * GL, C], [8, nb]]))

            def emit_P(g, p, s, nb):
                """transposed potential-add + masked state update + bp emit."""
                # smx = m + x^T[t]  (Pool, free nb)
                nc.gpsimd.tensor_tensor(
                    out=smx[g][:, 0:nb],
                    in0=bass.AP(mb[g], 0, [[2 * GL, C], [1, nb]]),
                    in1=xTr[g][p][:, s * GL:s * GL + nb], op=Alu.add)
                nc.vector.copy_predicated(
                    out=mbx[g].ap()[:, 0:nb],
                    mask=bass.AP(mTr[g][p], s * GL, [[CT * GL, C], [1, nb]]),
                    data=smx[g][:, 0:nb])
                # bp^T -> [8, C] -> masked br ring write
                tpb = psT.tile([GL, C], f32, tag="tpb")
                nc.tensor.transpose(
                    tpb[:], bass.AP(mb[g], GL, [[2 * GL, C], [1, GL]]), ident_sb)
                tpbs = sbT.tile([GL, C], f32, tag="tpbs")
                nc.scalar.copy(out=tpbs[:], in_=tpb[:])
                nc.vector.copy_predicated(
                    out=br[g][p][0:nb, s * C:(s + 1) * C],
                    mask=bass.AP(mr[g][p], s, [[CT, nb], [0, C]]),
                    data=tpbs[0:nb, :])

            def chunk_body(iv, p, n, first_skip):
                groups = [g for g in range(NG) if nb_of(g, n) > 0]
                for g in groups:
                    nc.sync.dma_start(out=xTr[g][p],
                                      in_=xT_d[g][:, bass.ds(iv * CT * GL, CT * GL)])
                    nc.sync.dma_start(out=mTr[g][p].ap(),
                                      in_=mT_d[g][:, bass.ds(iv * CT * GL, CT * GL)])
                    nc.sync.dma_start(out=mr[g][p].ap(),
                                      in_=mk_d[g][:, bass.ds(iv * CT, CT)])
                    # identity prefill of bp ring on Pool (iota over class dim)
                    nc.gpsimd.iota(
                        out=br[g][p].rearrange("b (s c) -> b s c", c=C),
                        pattern=[[0, CT], [1, C]], base=0, channel_multiplier=0,
                        allow_small_or_imprecise_dtypes=True)
                for s in range(CT):
                    if first_skip and s == 0:
                        continue
                    pend = []
                    for g in groups:
                        nb = nb_of(g, n)
                        emit_S(g, p, s, nb)
                        pend.append((g, nb))
                        if len(pend) >= 2:
                            pg, pnb = pend.pop(0)
                            emit_P(pg, p, s, pnb)
                    for (pg, pnb) in pend:
                        emit_P(pg, p, s, pnb)
                for g in groups:
                    nc.sync.dma_start(
                        out=bp_d[g].ap()[:, bass.ds(iv * CT * C, CT * C)],
                        in_=br[g][p])

            # ---- forward phases ----
            for (lo, hi, n) in phases:
                def mk(nn, is_first):
                    def body2(iv0, unroll):
                        for u in range(unroll):
                            chunk_body(iv0 + u, u % 2, nn,
                                       first_skip=(is_first and u == 0))
                    return body2
                if lo == 0:
                    tc.For_i_unrolled_general(
                        start=0, end=1, step=1,
                        unrollable_body=mk(n, True), max_unroll=1)
                    if hi > 1:
                        tc.For_i_unrolled_general(
                            start=1, end=hi, step=1,
                            unrollable_body=mk(n, False), max_unroll=2)
                else:
                    tc.For_i_unrolled_general(
                        start=lo, end=hi, step=1,
                        unrollable_body=mk(n, False), max_unroll=2)

            # ---- last tag: transpose final alpha^T back, reduce+argmax ----
            for g in range(NG):
                tpa = psT.tile([GL, C], f32, tag="tpb")
                nc.tensor.transpose(tpa[:], mbx[g].ap(), ident_sb)
                nc.scalar.copy(out=afin[g], in_=tpa[:])
                nc.vector.tensor_reduce(
                    out=mxg[g].ap(), in_=afin[g], axis=mybir.AxisListType.X,
                    op=Alu.max)
                nc.vector.max_index(
                    out=ltf[g].ap(),
                    in_max=bass.AP(mxg[g], 0, [[1, GL], [0, 8]]),
                    in_values=afin[g])
                nc.vector.tensor_copy(
                    out=afin[g][:, 0:1], in_=bass.AP(ltf[g], 0, [[8, GL], [1, 1]]))
                nc.sync.dma_start(out=lt_d.ap()[GL * g:GL * (g + 1), :],
                                  in_=afin[g][:, 0:1])
            nc.sync.dma_start(out=tagv[:, T - 1:T], in_=lt_d.ap())

            # ---- backtrace + one-hot emit (reversed chunks) ----
            for cc in range(NCH - 1, -1, -1):
                p = cc % 2
                for g in range(NG):
                    nc.sync.dma_start(
                        out=xb[p][GL * g:GL * (g + 1), :],
                        in_=bp_d[g].ap()[:, cc * CT * C:(cc + 1) * CT * C])
                for s in range(CT - 1, -1, -1):
                    u = cc * CT + s
                    if u == 0:
                        continue
                    nc.vector.scalar_tensor_tensor(
                        out=ohb, in0=iota32_sb, scalar=tagv[:, u:u + 1],
                        in1=xb[p][:, s * C:(s + 1) * C],
                        op0=Alu.is_equal, op1=Alu.mult,
                        accum_out=tagv[:, u - 1:u])
                nc.vector.tensor_tensor(
                    out=oe[p].rearrange("b (s c) -> b s c", c=C),
                    in0=bcast(iota32_sb, 1, CT),
                    in1=bcast(tagv[:, cc * CT:(cc + 1) * CT], 2, C),
                    op=Alu.is_equal)
                nc.sync.dma_start(out=out_d[:, cc * CT * C:(cc + 1) * CT * C],
                                  in_=oe[p])

    nc.compile()
    return nc


def _host_inputs(x, transitions, seq_lens, lanes):
    tt = np.ascontiguousarray(transitions.T).astype(np.float32)
    ident = np.eye(C, dtype=np.float32)
    iota32 = np.tile(np.arange(C, dtype=np.float32), (BLOC, 1))
    tgrid = np.arange(T)[None, :]
    in_maps = []
    for c in range(lanes.shape[0]):
        im = {"tt": tt, "ident": ident, "iota32": iota32}
        for g in range(NG):
            rows = [lanes[c][NG * r + g] for r in range(GL)]
            lx = x[rows]                                  # [GL, T, C]
            xT = np.ascontiguousarray(
                lx.transpose(2, 1, 0).reshape(C, T * GL)).astype(np.float32)
            L = seq_lens[rows].astype(np.int64)
            mk = (tgrid < L[:, None]).astype(np.int8)      # [GL, T]
            mT = np.ascontiguousarray(
                np.broadcast_to(mk.T[None, :, :], (C, T, GL)).reshape(C, T * GL))
            im[f"xT{g}"] = xT
            im[f"mT{g}"] = mT
            im[f"mask{g}"] = np.ascontiguousarray(mk)
        in_maps.append(im)
    return in_maps


TRACE = False
LAST_RESULT = None


def kernel(x, transitions, seq_lens):
    global LAST_RESULT
    from concourse.bass_utils import run_bass_kernel_spmd

    x = np.asarray(x, dtype=np.float32)
    transitions = np.asarray(transitions, dtype=np.float32)
    seq_lens = np.asarray(seq_lens)
    lanes = _assign_lanes(seq_lens)
    phases = _phase_schedule(seq_lens, lanes)
    key = tuple(phases)
    if key not in _prog_cache:
        _prog_cache[key] = build_program(phases)
    nc = _prog_cache[key]
    in_maps = _host_inputs(x, transitions, seq_lens, lanes)
    res = run_bass_kernel_spmd(nc, in_maps, list(range(NCORES)), trace=TRACE)
    LAST_RESULT = res
    out = np.empty((B, T, C), np.float32)
    for c in range(NCORES):
        oc = res.results[c]["out"].reshape(BLOC, T, C)
        for g in range(NG):
            for r in range(GL):
                out[lanes[c][NG * r + g]] = oc[GL * g + r]
    return out
